# revision 33
# baseline (speedup 1.0000x reference)
"""Fused QKV + RMSNorm + RoPE + self-attention kernel for Trainium2.

Sharding: tensor-parallel over heads. 16 heads / 8 cores = 2 heads per core.
Each core computes qkv projection for its 2 heads (column-parallel on the
3*dim output), per-head RMSNorm/RoPE/attention locally, and writes its
unnormalized AV accumulator [B, HPC, D, N] (bf16) plus softmax denominators.
The host divides, transposes to token-major and concatenates head slices
(the output projection is absent, so the "all-gather" is a host-side concat).

Host-side weight preprocessing:
  - the reference layout interleaves q/k/v at stride 3 per (head, dim):
    row = h*384 + d*3 + j.  We de-interleave by permuting w_qkv rows.
  - q/k head-dims are permuted even-first ([0,2,..,126,1,3,..,127]) so the
    interleaved RoPE becomes rotate-half style.  Scores q.k are invariant
    under a common permutation of q and k head-dims (RMSNorm too), and v is
    left unpermuted, so the final output is unchanged.
  - x / w / cos / sin are pre-tiled host-side so that every device DMA is
    contiguous per partition (cheap HWDGE descriptor generation).

Compute dtype: bf16 matmuls with fp32 accumulation; softmax/statistics fp32.
"""

import sys

sys.path.insert(0, "/opt/trn_rl_repo")

import numpy as np
import ml_dtypes

import concourse.bass as bass
import concourse.mybir as mybir
import concourse.tile as tile
from concourse import bacc
from concourse.masks import make_identity

B = 2
SEQ = 2048
DIM = 2048
NHEADS = 16
HEAD_DIM = 128
NCORES = 8
HPC = NHEADS // NCORES  # heads per core = 2
EPS = 1e-6
SCALE = float(HEAD_DIM) ** -0.5
P = 128  # partitions

F32 = mybir.dt.float32
BF16 = mybir.dt.bfloat16
I32 = mybir.dt.int32

QG = 512  # q tokens per attention inner group


def build_nc(seq=SEQ, batches=B):
    """Build the SPMD per-core graph. Same graph runs on all 8 cores."""
    tokens = batches * seq
    ntb = seq // P  # token tiles per batch (16)
    ntb_total = tokens // P  # 32
    kc_n = DIM // P  # contraction chunks for qkv projection (16)
    fpc = 3 * HPC * HEAD_DIM  # per-core projection output features = 768
    qg_per = seq // QG  # q groups per (b, h) (4)
    gq = QG // P  # 128-tiles per q group (4)

    nc = bacc.Bacc(None, target_bir_lowering=False)

    # host-pretiled layouts (all contiguous per partition):
    #   xt[p, t, kc, j]   = x[token t*128+j, dim kc*128+p]
    #   wt[p, kc, f]      = w[dim kc*128+p, feature f]
    #   cs/sc[p, ti, d]   = table[token ti*128+p, d]
    xt_ext = nc.declare_dram_parameter("xt", [P, ntb_total * kc_n * P], BF16, isOutput=False)
    wt_ext = nc.declare_dram_parameter("wt", [P, kc_n * fpc], BF16, isOutput=False)
    bias_ext = nc.declare_dram_parameter("bias", [P, fpc], F32, isOutput=False)
    cs_ext = nc.declare_dram_parameter("cs", [P, ntb * P], BF16, isOutput=False)
    sc_ext = nc.declare_dram_parameter("sc", [P, ntb * P], BF16, isOutput=False)
    av_ext = nc.declare_dram_parameter(
        "av", [batches, HPC, seq // QG, HEAD_DIM, QG], BF16, isOutput=True
    )
    sums_ext = nc.declare_dram_parameter(
        "sums", [batches, HPC, P, ntb], F32, isOutput=True
    )

    add = mybir.AluOpType.add
    sub = mybir.AluOpType.subtract
    mul = mybir.AluOpType.mult

    with tile.TileContext(nc) as tc:
        with (
            tc.tile_pool(name="consts", bufs=1) as consts,
            tc.tile_pool(name="persist", bufs=1) as persist,
        ):
            ones_col = consts.tile([P, 1], BF16, tag="ones")
            nc.vector.memset(ones_col[:], 1.0)
            ident = consts.tile([P, P], BF16, tag="ident")
            make_identity(nc, ident[:])


            wt_sb = consts.tile([P, kc_n, fpc], BF16, tag="wt")
            wt_r = wt_ext[:].rearrange("p (kc f) -> p kc f", kc=kc_n)
            bias_sb = consts.tile([P, fpc], F32, tag="bias")
            cs_sb = consts.tile([P, ntb, P], BF16, tag="cs")
            sc_sb = consts.tile([P, ntb, P], BF16, tag="sc")

            def emit_const_dmas():
                # weights stream per-chunk, split across the scalar HWDGE
                # ring (first half, consumed first) and the gpsimd SWDGE ring
                # (second half — its ~6us Q7 warmup overlaps the early
                # chunks).  Per-chunk DMAs cost ~0.7us of descriptor-gen
                # each, so a single ring cannot keep up with the matmul
                # stream's 0.35us/chunk consumption rate.
                for kc in range(kc_n // 2):
                    nc.scalar.dma_start(out=wt_sb[:, kc, :], in_=wt_r[:, kc, :])
                for kc in range(kc_n // 2, kc_n):
                    nc.gpsimd.dma_start(out=wt_sb[:, kc, :], in_=wt_r[:, kc, :])
                nc.scalar.dma_start(out=bias_sb[:], in_=bias_ext[:])

            def emit_rope_dmas():
                # cos/sin ride the gpsimd SWDGE queue so the sync ring stays
                # clear for the x-tile stream; the Q7 warmup overlaps the
                # projection cold start and they are first read at ~12 us
                cs_r = cs_ext[:].rearrange("p (ti d) -> p ti d", ti=ntb)
                sc_r = sc_ext[:].rearrange("p (ti d) -> p ti d", ti=ntb)
                nc.gpsimd.dma_start(out=cs_sb[:], in_=cs_r)
                nc.gpsimd.dma_start(out=sc_sb[:], in_=sc_r)

            # persistent per-(batch, local-head) attention operands
            # qT/kT feature-major: [d, tile, tok]; v token-major: [tok, chunk, d]
            qT = {}
            kT = {}
            vv = {}
            sums_sb = {}
            for b in range(batches):
                for hl in range(HPC):
                    qT[(b, hl)] = persist.tile([P, ntb, P], BF16, tag=f"qT{b}_{hl}", name=f"qT{b}_{hl}")
                    kT[(b, hl)] = persist.tile([P, ntb, P], BF16, tag=f"kT{b}_{hl}", name=f"kT{b}_{hl}")
                    vv[(b, hl)] = persist.tile([P, ntb, P], BF16, tag=f"v{b}_{hl}", name=f"v{b}_{hl}")
                    sums_sb[(b, hl)] = persist.tile(
                        [P, ntb], F32, tag=f"sums{b}_{hl}", name=f"sums{b}_{hl}"
                    )

            # Phase 1 (projection+norm+rope) and phase 2 (attention) share
            # pools and are emitted interleaved per batch, so batch b+1's
            # PE-heavy projection overlaps batch b's ACT-heavy softmax.
            with (
                tc.tile_pool(name="p1", bufs=2) as p1,
                tc.tile_pool(name="p1s", bufs=2) as p1s,
                tc.tile_pool(name="p2", bufs=2) as p2,
                tc.tile_pool(name="psp2", bufs=1, space="PSUM") as psp2,
            ):
                xt_r = xt_ext[:].rearrange(
                    "p (t kc j) -> p t kc j", t=ntb_total, kc=kc_n
                )
                pools = {}

                def phase1_tile(b_idx, ti):
                    t = b_idx * ntb + ti
                    # bufs=3: with 2, tile t+2's x-load can only ISSUE once
                    # tile t's matmuls release the slot, putting the DMA
                    # latency on the critical path during the cold start
                    x_tile = p1.tile(
                        [P, kc_n, P], BF16, tag="x", bufs=3, name="x_tile"
                    )
                    qc = kc_n // 4
                    if b_idx == 0 and ti == 0:
                        # tile 0 streams as quarters so the first matmul's
                        # chunk lands ASAP; later tiles use one DMA each
                        # (descriptor-gen is ~0.7us per dma_start)
                        for xq in range(4):
                            nc.sync.dma_start(
                                out=x_tile[:, xq * qc : (xq + 1) * qc, :],
                                in_=xt_r[:, t, xq * qc : (xq + 1) * qc, :],
                            )
                            if xq == 0:
                                emit_const_dmas()
                            if xq == 3:
                                emit_rope_dmas()
                    else:
                        nc.sync.dma_start(
                            out=x_tile[:], in_=xt_r[:, t, :, :]
                        )
                    ps_a = pools["psp1"].tile([P, 512], F32, tag="psA", bufs=2, name="ps_a")
                    ps_b = pools["psp1"].tile([P, 256], F32, tag="psB", bufs=1, name="ps_b")
                    for kc in range(kc_n):
                        st = kc == 0
                        sp = kc == kc_n - 1
                        nc.tensor.matmul(
                            ps_a[:],
                            x_tile[:, kc, :],
                            wt_sb[:, kc, 0:512],
                            start=st,
                            stop=sp,
                        )
                        nc.tensor.matmul(
                            ps_b[:],
                            x_tile[:, kc, :],
                            wt_sb[:, kc, 512:768],
                            start=st,
                            stop=sp,
                        )
                    # evac + bias add; qkv in bf16 so downstream DVE runs 2x
                    qkv_sb = p1.tile([P, fpc], BF16, tag="qkv")
                    nc.vector.tensor_tensor(
                        qkv_sb[:, 0:512], ps_a[:], bias_sb[:, 0:512], add
                    )
                    nc.vector.tensor_tensor(
                        qkv_sb[:, 512:768], ps_b[:], bias_sb[:, 512:768], add
                    )

                    # rms stats for the 4 q/k blocks (sq is a dummy out).
                    # ACT is idle while batch 0 projects (no softmax yet), so
                    # b0 stats run there; b1 stats stay on DVE.
                    ms = p1s.tile([P, 4], F32, tag="ms")
                    sq = p1s.tile([P, P], BF16, tag="sq", bufs=1)
                    for blk in range(4):
                        xb = qkv_sb[:, blk * P : (blk + 1) * P]
                        if b_idx == 0:
                            nc.scalar.activation(
                                out=sq[:],
                                in_=xb,
                                func=mybir.ActivationFunctionType.Square,
                                accum_out=ms[:, blk : blk + 1],
                            )
                        else:
                            nc.vector.scalar_tensor_tensor(
                                sq[:], xb, 1.0, xb, mul, mul,
                                accum_out=ms[:, blk : blk + 1],
                            )
                    # rstd = 1/sqrt(ms/128 + eps) via bit-trick + one Newton
                    # step on DVE (keeps ACT exp-only)
                    aa = p1s.tile([P, 4], F32, tag="aa")
                    nc.vector.tensor_scalar(
                        aa[:], ms[:], 1.0 / HEAD_DIM, EPS, mul, add
                    )
                    y0i = p1s.tile([P, 4], I32, tag="y0i")
                    nc.vector.tensor_scalar(
                        y0i[:], aa[:].bitcast(I32), 1, None,
                        mybir.AluOpType.logical_shift_right,
                    )
                    nc.vector.tensor_scalar(
                        y0i[:], y0i[:], -1, 0x5F3759DF, mul, add
                    )
                    y0 = y0i[:].bitcast(F32)
                    t1 = p1s.tile([P, 4], F32, tag="t1")
                    nc.vector.tensor_tensor(t1[:], y0, y0, mul)
                    nc.vector.scalar_tensor_tensor(
                        t1[:], t1[:], -0.5, aa[:], mul, mul
                    )
                    rstd = p1s.tile([P, 4], F32, tag="rstd")
                    nc.vector.scalar_tensor_tensor(
                        rstd[:], t1[:], 1.5, y0, add, mul
                    )
                    # second Newton step for accuracy
                    nc.vector.tensor_tensor(t1[:], rstd[:], rstd[:], mul)
                    nc.vector.scalar_tensor_tensor(
                        t1[:], t1[:], -0.5, aa[:], mul, mul
                    )
                    nc.vector.scalar_tensor_tensor(
                        rstd[:], t1[:], 1.5, rstd[:], add, mul
                    )

                    # wide rope over all 4 q/k blocks at once:
                    #   m1 = qk * [c|s]x4   m2 = qk * [-s|c]x4
                    #   roped = per-block [m1_lo - m2_lo_pair ...] via one
                    #   strided subtract (4D APs)
                    csb = cs_sb[:, ti, :]
                    scb = sc_sb[:, ti, :]
                    cs_rep = bass.AP(
                        tensor=csb.tensor, offset=csb.offset,
                        ap=[list(csb.ap[0]), [0, 4], [1, P]],
                    )
                    sc_rep = bass.AP(
                        tensor=scb.tensor, offset=scb.offset,
                        ap=[list(scb.ap[0]), [0, 4], [1, P]],
                    )
                    m12 = p1.tile([P, 2, 4, P], BF16, tag="m12")
                    qk_in = qkv_sb[:, 0:512].rearrange("p (r c) -> p r c", r=4)
                    nc.vector.tensor_tensor(m12[:, 0], qk_in, cs_rep, mul)
                    nc.vector.tensor_tensor(m12[:, 1], qk_in, sc_rep, mul)
                    mb = m12[:]
                    # a: [x1c | x2c] per block; b: [x2s | -x1s] per block
                    a_ap = bass.AP(
                        tensor=mb.tensor, offset=mb.offset,
                        ap=[list(mb.ap[0]), [P, 4], [576, 2], [1, 64]],
                    )
                    b_ap = bass.AP(
                        tensor=mb.tensor, offset=mb.offset + 64,
                        ap=[list(mb.ap[0]), [P, 4], [448, 2], [1, 64]],
                    )
                    roped = p1.tile([P, 4, 2, 64], BF16, tag="roped")
                    nc.vector.tensor_tensor(roped[:], a_ap, b_ap, sub)

                    # normalize (q and k) + transpose via PE into
                    # feature-major persistent tiles
                    rview = roped[:].rearrange("p b a c -> p (b a c)")
                    norm_sb = p1.tile([P, 512], BF16, tag="norm")
                    for blk in range(4):
                        c0 = blk * P
                        nc.vector.tensor_scalar_mul(
                            norm_sb[:, c0 : c0 + P],
                            rview[:, c0 : c0 + P],
                            rstd[:, blk : blk + 1],
                        )
                    # transposes + v copies are deferred: emitted after the
                    # NEXT tile's matmuls so the PE never queues a transpose
                    # that waits on this tile's still-running DVE chain
                    def part_b():
                        for blk in range(4):
                            dest = qT if blk < 2 else kT
                            hl = blk % 2
                            tp = pools["psp1"].tile(
                                [P, P], BF16, tag="tp", bufs=1, name="tp"
                            )
                            nc.tensor.transpose(
                                tp[:],
                                norm_sb[:, blk * P : (blk + 1) * P],
                                ident[:],
                            )
                            nc.vector.tensor_copy(
                                dest[(b_idx, hl)][:, ti, :], tp[:]
                            )
                        for hl in range(HPC):
                            c0 = 512 + hl * P
                            nc.gpsimd.tensor_copy(
                                vv[(b_idx, hl)][:, ti, :], qkv_sb[:, c0 : c0 + P]
                            )

                    return part_b

                def emit_exp_pair(probsT, k_t, qs_ap, kc, alt=False):
                    if alt:
                        s_ps = pools["psp3"].tile(
                            [P, 1024], F32, tag="spsB2", bufs=1, name="s_ps2"
                        )
                    else:
                        s_ps = psp2.tile(
                            [P, 1024], F32, tag="spsB", bufs=1, name="s_ps"
                        )
                    nc.tensor.matmul(
                        s_ps[:, 0:512], k_t[:, kc, :], qs_ap,
                        start=True, stop=True,
                    )
                    nc.tensor.matmul(
                        s_ps[:, 512:1024], k_t[:, kc + 1, :], qs_ap,
                        start=True, stop=True,
                    )
                    nc.scalar.activation(
                        out=probsT[:, kc : kc + 2, :],
                        in_=s_ps[:],
                        func=mybir.ActivationFunctionType.Exp,
                        scale=SCALE,
                    )

                def emit_half_fold(probsT, half):
                    # fold chunks [8h, 8h+8) down to [P, 512]
                    cur = probsT[:, 8 * half : 8 * half + 8, :].rearrange(
                        "p a b -> p (a b)"
                    )
                    width = 8 * QG
                    lvl = 1
                    while width > QG:
                        width //= 2
                        nxt = p2.tile(
                            [P, width], BF16, tag=f"fold{lvl}",
                            bufs=(2 if width == QG else 1), name="hfold",
                        )
                        nc.vector.tensor_tensor(
                            nxt[:], cur[:, 0:width],
                            cur[:, width : 2 * width], add,
                        )
                        cur = nxt[:]
                        lvl += 1
                    return cur

                def emit_folds(probsT):
                    cur = probsT[:].rearrange("p a b -> p (a b)")
                    width = ntb * QG
                    lvl = 0
                    while width > QG:
                        width //= 2
                        nxt = p2.tile(
                            [P, width], BF16, tag=f"fold{lvl}",
                            bufs=(2 if width == QG else 1), name="fold",
                        )
                        nc.vector.tensor_tensor(
                            nxt[:],
                            cur[:, 0:width],
                            cur[:, width : 2 * width],
                            add,
                        )
                        cur = nxt[:]
                        lvl += 1
                    return cur

                def emit_scol(b, hl, qg, sums, tail_scol, on_act=False):
                    # partition-sums go through the (phase-1) tp bank via a
                    # f32 view — or a recycled bank in the tail.  The LDW of
                    # the 128-col stationary hides behind neighboring matmuls.
                    if tail_scol:
                        st = pools["psp3"].tile(
                            [P, 4], F32, tag="scolT", bufs=1, name="scol_t"
                        )
                        scol = st[:]
                    else:
                        tp = pools["psp1"].tile(
                            [P, P], BF16, tag="tp", bufs=1, name="tp_s"
                        )
                        scol = tp[:, 0:8].bitcast(F32)
                    for qs in range(gq):
                        nc.tensor.matmul(
                            scol[:, qs : qs + 1],
                            sums[:, qs * P : (qs + 1) * P],
                            ones_col[:],
                            start=True,
                            stop=True,
                            skip_group_check=True,
                        )
                    dst = sums_sb[(b, hl)][:, qg * gq : (qg + 1) * gq]
                    if on_act:
                        nc.scalar.activation(
                            out=dst, in_=scol[:, 0:gq],
                            func=mybir.ActivationFunctionType.Copy,
                        )
                    else:
                        nc.vector.tensor_copy(dst, scol[:, 0:gq])
                    if qg == qg_per - 1:
                        nc.sync.dma_start(
                            out=sums_ext[b, hl, :, :], in_=sums_sb[(b, hl)][:]
                        )

                def emit_av_evac(b, hl, qg, av_ps, act_evac):
                    # evac AV (psum->sbuf bf16), ship unnormalized to host
                    av_sb = p2.tile([P, QG], BF16, tag="avsb", name="av_sb")
                    if act_evac:
                        nc.scalar.activation(
                            out=av_sb[:],
                            in_=av_ps[:],
                            func=mybir.ActivationFunctionType.Copy,
                        )
                    else:
                        nc.vector.tensor_copy(av_sb[:], av_ps[:])
                    nc.sync.dma_start(
                        out=av_ext[b, hl, qg, :, :],
                        in_=av_sb[:],
                    )

                def phase2_qgroup(b, hl, qg):
                    """Middle-phase qgroup: scores+exp, folds, AV, evac.
                    Returns the deferred denominator closure."""
                    q_t = qT[(b, hl)]
                    k_t = kT[(b, hl)]
                    v_t = vv[(b, hl)]
                    qs_ap = q_t[:, qg * gq : (qg + 1) * gq, :]
                    probsT = p2.tile([P, ntb, QG], BF16, tag="probsT", bufs=3, name="probsT")
                    # big(1024)/small(512) exp ping-pong — the small slot
                    # keeps ACT busy while PE refills the single big slot
                    kc = 0
                    while kc < ntb:
                        if kc % 3 == 0 and kc + 1 < ntb:
                            emit_exp_pair(probsT, k_t, qs_ap, kc)
                            kc += 2
                        else:
                            s_ps = psp2.tile(
                                [P, 512], F32, tag="spsS", bufs=1,
                                name="s_ps_s",
                            )
                            nc.tensor.matmul(
                                s_ps[:], k_t[:, kc, :], qs_ap,
                                start=True, stop=True,
                            )
                            nc.scalar.activation(
                                out=probsT[:, kc, :],
                                in_=s_ps[:],
                                func=mybir.ActivationFunctionType.Exp,
                                scale=SCALE,
                            )
                            kc += 1
                    sums = emit_folds(probsT)
                    av_ps = psp2.tile(
                        [P, QG], F32, tag="av", bufs=1, name="av_ps"
                    )
                    for kc in range(ntb):
                        nc.tensor.matmul(
                            av_ps[:],
                            v_t[:, kc, :],
                            probsT[:, kc, :],
                            start=(kc == 0),
                            stop=(kc == ntb - 1),
                        )
                    emit_av_evac(b, hl, qg, av_ps, act_evac=(b == 0))

                    def scol_tail():
                        emit_scol(b, hl, qg, sums, tail_scol=False)

                    return scol_tail

                def tail_scores_av(b, hl, qg, prev, last):
                    """Tail-phase scores+exp for (b,hl,qg) with the PREVIOUS
                    unit's AV chunk-matmuls interleaved between exp pairs, so
                    the PE (not the 2-slot exp psum ping-pong) paces the tail.
                    Returns (probsT, half1, av_ps_prev)."""
                    q_t = qT[(b, hl)]
                    k_t = kT[(b, hl)]
                    qs_ap = q_t[:, qg * gq : (qg + 1) * gq, :]
                    probsT = p2.tile([P, ntb, QG], BF16, tag="probsT", bufs=3, name="probsT")
                    half1 = None
                    av_ps = None
                    if prev is not None:
                        av_ps = psp2.tile(
                            [P, QG], F32, tag="av", bufs=1, name="av_ps"
                        )
                        pv = vv[(prev["b"], prev["hl"])]
                        pprobs = prev["probsT"]
                    for pr in range(ntb // 2):
                        emit_exp_pair(
                            probsT, k_t, qs_ap, 2 * pr, alt=(pr % 2 == 1)
                        )
                        if prev is not None:
                            for kk in (2 * pr, 2 * pr + 1):
                                nc.tensor.matmul(
                                    av_ps[:],
                                    pv[:, kk, :],
                                    pprobs[:, kk, :],
                                    start=(kk == 0),
                                    stop=(kk == ntb - 1),
                                )
                        if last and pr == ntb // 4 - 1:
                            half1 = emit_half_fold(probsT, 0)
                    return probsT, half1, av_ps

                # interleaved emission: batch b's projection tiles are woven
                # between batch b-1's attention qgroups so PE-heavy and
                # ACT-heavy work stay concurrently available to the scheduler
                p2_units = {
                    b: [(b, hl, qg) for qg in range(qg_per) for hl in range(HPC)]
                    for b in range(batches)
                }
                pending_b = None
                pending_s = None

                def emit_tile(b_idx, ti):
                    nonlocal pending_b
                    nxt = phase1_tile(b_idx, ti)
                    if pending_b is not None:
                        pending_b()
                    pending_b = nxt

                def emit_qgroup(u):
                    nonlocal pending_s
                    nxt = phase2_qgroup(*u)
                    if pending_s is not None:
                        pending_s()
                    pending_s = nxt

                with tc.tile_pool(name="psp1", bufs=1, space="PSUM") as psp1:
                    pools["psp1"] = psp1
                    for ti in range(ntb):
                        emit_tile(0, ti)
                    for b in range(1, batches):
                        prev = p2_units[b - 1]
                        ratio = max(1, ntb // max(1, len(prev)))
                        pi = 0
                        for ti in range(ntb):
                            emit_tile(b, ti)
                            if (ti + 1) % ratio == 0 and pi < len(prev):
                                emit_qgroup(prev[pi])
                                pi += 1
                        while pi < len(prev):
                            emit_qgroup(prev[pi])
                            pi += 1
                    if pending_b is not None:
                        pending_b()
                        pending_b = None
                    if pending_s is not None:
                        pending_s()
                        pending_s = None
                with tc.tile_pool(name="psp3", bufs=1, space="PSUM") as psp3:
                    pools["psp3"] = psp3
                    # tail: batch-1 attention only, software-pipelined one
                    # unit deep.  Per iteration: folds(g-1) first (DVE queue
                    # unblocked early), then scores(g) with AV(g-1)
                    # interleaved between exp pairs, then evac+denominator
                    # for g-1.  The last two units evacuate on ACT (its exps
                    # are done by then; DVE still folds).
                    tail_units = p2_units[batches - 1]
                    nunit = len(tail_units)
                    pend = None
                    for ui, u in enumerate(tail_units):
                        is_last = ui == nunit - 1
                        if pend is not None:
                            pend["sums"] = emit_folds(pend["probsT"])
                        probsT, half1, av_prev = tail_scores_av(
                            u[0], u[1], u[2], pend, last=is_last
                        )
                        if pend is not None:
                            # the final units evacuate on ACT (its exps are
                            # done by then); earlier units stay on DVE so ACT
                            # (~8.9us/qgroup with an evac) never out-paces
                            # the PE period (~8.6us)
                            act = pend["ui"] >= nunit - 2
                            emit_av_evac(
                                pend["b"], pend["hl"], pend["qg"], av_prev,
                                act_evac=act,
                            )
                            emit_scol(
                                pend["b"], pend["hl"], pend["qg"],
                                pend["sums"], tail_scol=True, on_act=act,
                            )
                        pend = {
                            "ui": ui, "b": u[0], "hl": u[1], "qg": u[2],
                            "probsT": probsT, "half1": half1, "last": is_last,
                        }
                    # final flush: the last unit's own fold tail + straight AV
                    half2 = emit_half_fold(pend["probsT"], 1)
                    ff = p2.tile([P, QG], BF16, tag="foldF", bufs=1, name="ff")
                    nc.vector.tensor_tensor(ff[:], pend["half1"], half2, add)
                    av_ps = psp2.tile([P, QG], F32, tag="av", bufs=1, name="av_ps")
                    v_t = vv[(pend["b"], pend["hl"])]
                    for kc in range(ntb):
                        nc.tensor.matmul(
                            av_ps[:],
                            v_t[:, kc, :],
                            pend["probsT"][:, kc, :],
                            start=(kc == 0),
                            stop=(kc == ntb - 1),
                        )
                    emit_av_evac(
                        pend["b"], pend["hl"], pend["qg"], av_ps, act_evac=True
                    )
                    emit_scol(
                        pend["b"], pend["hl"], pend["qg"], ff[:],
                        tail_scol=True, on_act=True,
                    )

    nc.compile()
    return nc


def prep_inputs(x, w_qkv, b_qkv, cos, sin):
    """Build per-core input maps (host-side sharding + re-tiling)."""
    bf16 = ml_dtypes.bfloat16
    batches, seq, dim = x.shape
    ntb_total = batches * seq // P
    kc_n = dim // P
    ntb = seq // P
    # xt[p, t, kc, j] = x[tok = t*128 + j, dim = kc*128 + p]
    xf = x.reshape(batches * seq, dim)
    xt = np.ascontiguousarray(
        xf.reshape(ntb_total, P, kc_n, P).transpose(3, 0, 2, 1)
        .reshape(P, ntb_total * kc_n * P).astype(bf16)
    )
    cosf = cos.astype(np.float32)
    sinf = sin.astype(np.float32)
    csf = np.concatenate([cosf, sinf], axis=1).astype(bf16)  # [seq, 128]
    scf = np.concatenate([-sinf, cosf], axis=1).astype(bf16)
    # cs[p, ti, d] = table[ti*128 + p, d]
    cst = np.ascontiguousarray(
        csf.reshape(ntb, P, P).transpose(1, 0, 2).reshape(P, ntb * P)
    )
    sct = np.ascontiguousarray(
        scf.reshape(ntb, P, P).transpose(1, 0, 2).reshape(P, ntb * P)
    )
    dperm = np.concatenate([np.arange(0, HEAD_DIM, 2), np.arange(1, HEAD_DIM, 2)])
    dnat = np.arange(HEAD_DIM)
    in_maps = []
    for c in range(NCORES):
        h0, h1 = HPC * c, HPC * c + 1
        idx = np.concatenate(
            [
                h0 * 384 + dperm * 3 + 0,
                h1 * 384 + dperm * 3 + 0,
                h0 * 384 + dperm * 3 + 1,
                h1 * 384 + dperm * 3 + 1,
                h0 * 384 + dnat * 3 + 2,
                h1 * 384 + dnat * 3 + 2,
            ]
        )
        fpc = len(idx)
        w = w_qkv[idx, :].T.astype(bf16)  # [DIM, fpc]
        # wt[p, kc, f] = w[kc*128 + p, f]
        wt = np.ascontiguousarray(
            w.reshape(kc_n, P, fpc).transpose(1, 0, 2).reshape(P, kc_n * fpc)
        )
        bb = np.ascontiguousarray(
            np.broadcast_to(b_qkv[idx].astype(np.float32)[None, :], (P, fpc))
        )
        in_maps.append(
            {"xt": xt, "wt": wt, "bias": bb, "cs": cst, "sc": sct}
        )
    return in_maps


_CACHED = {}


def _get_nc(seq, batches):
    key = (seq, batches)
    if key not in _CACHED:
        _CACHED[key] = build_nc(seq, batches)
    return _CACHED[key]


def run(x, w_qkv, b_qkv, cos, sin, trace=False):
    from concourse.bass_utils import run_bass_kernel_spmd

    batches, seq, _ = x.shape
    nc = _get_nc(seq, batches)
    in_maps = prep_inputs(x, w_qkv, b_qkv, cos, sin)
    res = run_bass_kernel_spmd(
        nc, in_maps, core_ids=list(range(NCORES)), trace=trace
    )
    out = np.empty((batches, seq, NCORES * HPC * HEAD_DIM), dtype=np.float32)
    for c in range(NCORES):
        # av: [B, HPC, qg, D, QG] bf16; sums: [B, HPC, P, ntb] f32
        av = np.asarray(res.results[c]["av"], dtype=np.float32)
        sums = res.results[c]["sums"]
        for b in range(batches):
            for hl in range(HPC):
                s = sums[b, hl].T.reshape(seq)  # q = j*128 + p
                h = (HPC * c + hl) * HEAD_DIM
                avf = av[b, hl].transpose(1, 0, 2).reshape(HEAD_DIM, seq)
                out[b, :, h : h + HEAD_DIM] = (avf / s[None, :]).T
    return out, res


def kernel(x, w_qkv, b_qkv, cos, sin):
    out, _ = run(
        np.asarray(x),
        np.asarray(w_qkv),
        np.asarray(b_qkv),
        np.asarray(cos),
        np.asarray(sin),
        trace=False,
    )
    return out
